# revision 1
# baseline (speedup 1.0000x reference)
"""NTM addressing head (nn_HeadBase) Trainium2 Bass kernel.

Full-input contract: kernel(**inputs) takes the unsharded [256, ...] arrays,
shards batch-dim across 8 NeuronCores (pure data parallel), runs one SPMD Bass
program per core, and gathers the full [256, 4096] output.

Per-core layout (B=32 batches, N=4096, M=64):
  memory[b] is streamed as one [128, 2048] SBUF tile per batch with
  n = p*32 + r (partition p, free = (r, m)); 4 KB contiguous per partition.
  Phase A (per batch): prod = mem * k_bcast (DVE), dot = reduce over m
  (DVE), sq = mem^2 (ACT), ssq = reduce (DVE).
  Phase B (all batches fused as [128, 32b*32r] tiles): cosine sim, softmax
  (no max-subtract needed: beta*sim in (-1,1)), gated interpolation, 3-tap
  circular shift via shifted APs + partition-carry fixups, pow via exp/ln,
  final normalize.  Per-batch scalars are broadcast to [128, B] via K=1
  ones-matmuls on the TensorEngine.
"""

import numpy as np

B_FULL, N, M = 256, 4096, 64
NCORES = 8
B = B_FULL // NCORES   # 32 batches per core
P = 128                # SBUF partitions
R = N // P             # 32 rows per partition; n = p*R + r

_NC_CACHE = {}


def _build_body(nc, out_ap, ins):
    """Emit the kernel IR. ins: dict name->AP of DRAM inputs, out_ap: DRAM out."""
    from contextlib import ExitStack

    import concourse.bass as bass
    import concourse.tile as tile
    from concourse import mybir

    f32 = mybir.dt.float32
    Alu = mybir.AluOpType
    Act = mybir.ActivationFunctionType
    Ax = mybir.AxisListType
    AP = bass.AP

    mem_ap = ins["memory"]   # [B, N, M]
    k_ap = ins["k"]          # [B, M]
    beta_ap = ins["beta"]    # [B, 1]
    pw_ap = ins["prev_w"]    # [B, N]
    g_ap = ins["g"]          # [B, 1]
    s_ap = ins["s"]          # [B, 3]
    gam_ap = ins["gamma"]    # [B, 1]

    def bcast_inner(ap2d, n):
        # [P, C] -> [P, C, n] with 0-stride inner dim
        return AP(ap2d.tensor, ap2d.offset, list(ap2d.ap) + [[0, n]])

    def bcast_mid(ap2d, n):
        # [P, C] -> [P, n, C] with 0-stride middle dim
        a = list(ap2d.ap)
        return AP(ap2d.tensor, ap2d.offset, [a[0], [0, n], a[1]])

    def row1(ap1d):
        # prepend a unit partition dim to a 1-d AP
        return AP(ap1d.tensor, ap1d.offset, [[0, 1]] + list(ap1d.ap))

    with tile.TileContext(nc) as tc, ExitStack() as ctx:
        singles = ctx.enter_context(tc.tile_pool(name="singles", bufs=1))
        mem_pool = ctx.enter_context(tc.tile_pool(name="mem", bufs=3))
        prod_pool = ctx.enter_context(tc.tile_pool(name="prod", bufs=2))
        big = ctx.enter_context(tc.tile_pool(name="big", bufs=1))
        ps = ctx.enter_context(tc.tile_pool(name="ps", bufs=2, space="PSUM"))
        ps_big = ctx.enter_context(tc.tile_pool(name="psbig", bufs=1, space="PSUM"))

        # ---- setup: constants, per-batch scalar rows on partition 0 ----
        ones_col = singles.tile([P, 1], f32, tag="ones_col")
        nc.vector.memset(ones_col, 1.0)
        ones_row = singles.tile([1, P], f32, tag="ones_row")
        nc.vector.memset(ones_row, 1.0)

        k_row = singles.tile([1, B * M], f32, tag="k_row")
        nc.sync.dma_start(out=k_row, in_=row1(k_ap.rearrange("b m -> (b m)")))
        b_row = singles.tile([1, B], f32, tag="b_row")
        nc.sync.dma_start(out=b_row, in_=row1(beta_ap.rearrange("b one -> (b one)")))
        g_row = singles.tile([1, B], f32, tag="g_row")
        nc.sync.dma_start(out=g_row, in_=row1(g_ap.rearrange("b one -> (b one)")))
        gm_row = singles.tile([1, B], f32, tag="gm_row")
        nc.sync.dma_start(out=gm_row, in_=row1(gam_ap.rearrange("b one -> (b one)")))
        s_row = singles.tile([1, 3 * B], f32, tag="s_row")
        nc.sync.dma_start(out=s_row, in_=row1(s_ap.rearrange("b i -> (b i)")))
        # s_i as [1, B] strided views (stride 3)
        s_perm = s_row.rearrange("p (b i) -> p i b", i=3)
        s_v = [s_perm[:, i, :] for i in range(3)]

        # k broadcast to all partitions: kb[p, b*M+m] = k[b, m].
        # Matmult can carry only ONE sync-wait; touch k_row on DVE first so
        # both matmul deps (ones_row memset + k data) ride the DVE semaphore.
        k_row2 = prod_pool.tile([1, B * M], f32, tag="pr")
        nc.vector.tensor_copy(k_row2, k_row)
        kb_psum = ps_big.tile([P, B * M], f32, tag="kb_psum")
        for j in range(0, B * M, 512):
            nc.tensor.matmul(
                kb_psum[:, j : j + 512], ones_row, k_row2[:, j : j + 512],
                start=True, stop=True,
            )
        kb = singles.tile([P, B * M], f32, tag="kb")
        nc.scalar.copy(out=kb, in_=kb_psum)

        # knorm; bk = beta / knorm
        ksq_row = prod_pool.tile([1, B * M], f32, tag="pr")
        nc.vector.tensor_mul(ksq_row, k_row, k_row)
        ks_row = singles.tile([1, B], f32, tag="ks_row")
        nc.vector.tensor_reduce(
            out=ks_row, in_=ksq_row.rearrange("p (b m) -> p b m", m=M),
            axis=Ax.X, op=Alu.add,
        )
        kn_row = singles.tile([1, B], f32, tag="kn_row")
        nc.scalar.activation(out=kn_row, in_=ks_row, func=Act.Sqrt)
        rk_row = singles.tile([1, B], f32, tag="rk_row")
        nc.vector.reciprocal(out=rk_row, in_=kn_row)
        bk_row = singles.tile([1, B], f32, tag="bk_row")
        nc.vector.tensor_mul(bk_row, b_row, rk_row)

        # omg = 1 - g
        omg_row = singles.tile([1, B], f32, tag="omg_row")
        nc.vector.tensor_scalar(
            out=omg_row, in0=g_row, scalar1=-1.0, scalar2=1.0,
            op0=Alu.mult, op1=Alu.add,
        )

        # broadcast round 1: [bk, omg, s0, s1, s2, gamma] -> [P, 6*B]
        NSC = 6
        asm1 = singles.tile([1, NSC * B], f32, tag="asm1")
        for i, src in enumerate([bk_row, omg_row, s_v[0], s_v[1], s_v[2], gm_row]):
            nc.vector.tensor_copy(asm1[:, i * B : (i + 1) * B], src)
        bc1_ps = ps.tile([P, NSC * B], f32, tag="mm")
        nc.tensor.matmul(bc1_ps, ones_row, asm1, start=True, stop=True)
        BC1 = singles.tile([P, NSC * B], f32, tag="BC1")
        nc.scalar.copy(out=BC1, in_=bc1_ps)
        BK = BC1[:, 0 * B : 1 * B]
        OMG = BC1[:, 1 * B : 2 * B]
        S0 = BC1[:, 2 * B : 3 * B]
        S1 = BC1[:, 3 * B : 4 * B]
        S2 = BC1[:, 4 * B : 5 * B]
        GAM = BC1[:, 5 * B : 6 * B]

        # prev_w big tile [P, B*R] in one permuted-AP DMA (128B inner runs)
        pw = big.tile([P, B * R], f32, tag="pw")
        nc.sync.dma_start(
            out=pw.rearrange("p (b r) -> p b r", r=R),
            in_=pw_ap.rearrange("b (p r) -> p b r", r=R),
        )

        # ---- phase A: stream memory in CB-batch chunks ----
        # multiply on GpSimd (otherwise idle), square on ACT, reduces on DVE.
        CB = 4  # batches per chunk
        dot = big.tile([P, B * R], f32, tag="dot")
        ssq = big.tile([P, B * R], f32, tag="ssq")
        for c in range(B // CB):
            b0 = c * CB
            mt = mem_pool.tile([P, CB * R * M], f32, tag="mt")
            nc.sync.dma_start(
                out=mt.rearrange("p (b f) -> p b f", b=CB),
                in_=mem_ap[b0 : b0 + CB].rearrange(
                    "b (p r) m -> p b (r m)", p=P
                ),
            )
            mt4 = mt.rearrange("p (b r m) -> p b r m", b=CB, m=M)
            pr = prod_pool.tile([P, CB * R * M], f32, tag="pr")
            pr4 = pr.rearrange("p (b r m) -> p b r m", b=CB, m=M)
            kbc = kb[:, b0 * M : (b0 + CB) * M]  # [P, CB*M]
            kb4 = AP(
                kbc.tensor, kbc.offset,
                [kbc.ap[0], [M, CB], [0, R], [1, M]],
            )
            nc.gpsimd.tensor_tensor(out=pr4, in0=mt4, in1=kb4, op=Alu.mult)
            nc.vector.tensor_reduce(
                out=dot[:, b0 * R : (b0 + CB) * R].rearrange(
                    "p (b r) -> p b r", b=CB),
                in_=pr4, axis=Ax.X, op=Alu.add,
            )
            # square mt in place (ACT); Tile orders it after the gpsimd mult
            nc.scalar.square(out=mt, in_=mt)
            nc.vector.tensor_reduce(
                out=ssq[:, b0 * R : (b0 + CB) * R].rearrange(
                    "p (b r) -> p b r", b=CB),
                in_=mt4, axis=Ax.X, op=Alu.add,
            )

        # ---- phase B ----
        def v3(t):
            return t.rearrange("p (b r) -> p b r", r=R)

        # rstd = 1/sqrt(ssq_avg)
        mn = big.tile([P, B * R], f32, tag="mn")
        nc.scalar.activation(out=mn, in_=ssq, func=Act.Sqrt)
        scr = prod_pool.tile([P, B * R], f32, tag="pr")
        nc.vector.reciprocal_approx_accurate(out=ssq, in_=mn, scratch=scr)

        # a = (8*beta/knorm) * dot_avg * rstd
        nc.vector.tensor_mul(dot, dot, ssq)
        nc.vector.tensor_mul(v3(dot), v3(dot), bcast_inner(BK, R))

        # e = exp(a)
        e = big.tile([P, B * R], f32, tag="e")
        nc.scalar.activation(out=e, in_=dot, func=Act.Exp)

        # denom per batch; gd = g/denom
        cs = singles.tile([P, B], f32, tag="cs")
        nc.vector.tensor_reduce(out=cs, in_=v3(e), axis=Ax.X, op=Alu.add)
        den_ps = ps.tile([1, B], f32, tag="mm")
        nc.tensor.matmul(den_ps, ones_col, cs, start=True, stop=True)
        rden_row = singles.tile([1, B], f32, tag="rden_row")
        nc.vector.reciprocal(out=rden_row, in_=den_ps)
        gd_row = singles.tile([1, B], f32, tag="gd_row")
        nc.vector.tensor_mul(gd_row, rden_row, g_row)
        gd_ps = ps.tile([P, B], f32, tag="mm")
        nc.tensor.matmul(gd_ps, ones_row, gd_row, start=True, stop=True)
        GD = singles.tile([P, B], f32, tag="GD")
        nc.scalar.copy(out=GD, in_=gd_ps)

        # wg = e*gd + pw*omg   (in place into e)
        nc.vector.tensor_mul(v3(e), v3(e), bcast_inner(GD, R))
        nc.vector.tensor_mul(v3(pw), v3(pw), bcast_inner(OMG, R))
        nc.vector.tensor_add(out=e, in0=e, in1=pw)

        # circular 3-tap shift: ws[n] = s1*wg[n] + s0*wg[n-1] + s2*wg[n+1]
        ws = big.tile([P, B * R], f32, tag="ws")
        ta = prod_pool.tile([P, B * R], f32, tag="pr")
        tb = prod_pool.tile([P, B * R], f32, tag="pr")
        wg3, ws3, ta3, tb3 = v3(e), v3(ws), v3(ta), v3(tb)
        nc.vector.tensor_mul(ws3, wg3, bcast_inner(S1, R))
        nc.vector.tensor_mul(ta3, wg3, bcast_inner(S0, R))
        nc.vector.tensor_mul(tb3, wg3, bcast_inner(S2, R))
        nc.vector.tensor_add(
            out=ws3[:, :, 1:R], in0=ws3[:, :, 1:R], in1=ta3[:, :, 0 : R - 1]
        )
        nc.vector.tensor_add(
            out=ws3[:, :, 0 : R - 1], in0=ws3[:, :, 0 : R - 1], in1=tb3[:, :, 1:R]
        )
        # partition carries: engines need 32-aligned start partitions, so the
        # +-1 partition rotation goes through small SBUF->SBUF DMAs.
        # tmp_dn[p] = ta[(p-1) mod P, :, R-1];  tmp_up[p] = tb[(p+1) mod P, :, 0]
        tmp_dn = singles.tile([P, B], f32, tag="tmp_dn")
        nc.sync.dma_start(out=tmp_dn[1:P, :], in_=ta3[0 : P - 1, :, R - 1 : R])
        nc.sync.dma_start(out=tmp_dn[0:1, :], in_=ta3[P - 1 : P, :, R - 1 : R])
        tmp_up = singles.tile([P, B], f32, tag="tmp_up")
        nc.sync.dma_start(out=tmp_up[0 : P - 1, :], in_=tb3[1:P, :, 0:1])
        nc.sync.dma_start(out=tmp_up[P - 1 : P, :], in_=tb3[0:1, :, 0:1])
        nc.vector.tensor_add(
            out=ws3[:, :, 0:1], in0=ws3[:, :, 0:1], in1=bcast_inner(tmp_dn, 1)
        )
        nc.vector.tensor_add(
            out=ws3[:, :, R - 1 : R], in0=ws3[:, :, R - 1 : R],
            in1=bcast_inner(tmp_up, 1),
        )

        # w_pow = ws ** gamma = exp(gamma * ln(ws))
        nc.scalar.activation(out=ws, in_=ws, func=Act.Ln)
        nc.vector.tensor_mul(ws3, ws3, bcast_inner(GAM, R))
        nc.scalar.activation(out=ws, in_=ws, func=Act.Exp)

        # normalize: out = w_pow / (sum + 1e-16)
        cs2 = singles.tile([P, B], f32, tag="cs2")
        nc.vector.tensor_reduce(out=cs2, in_=ws3, axis=Ax.X, op=Alu.add)
        d2_ps = ps.tile([1, B], f32, tag="mm")
        nc.tensor.matmul(d2_ps, ones_col, cs2, start=True, stop=True)
        d2_row = singles.tile([1, B], f32, tag="d2_row")
        nc.vector.tensor_scalar_add(out=d2_row, in0=d2_ps, scalar1=1e-16)
        rd2_row = singles.tile([1, B], f32, tag="rd2_row")
        nc.vector.reciprocal(out=rd2_row, in_=d2_row)
        rd2_ps = ps.tile([P, B], f32, tag="mm")
        nc.tensor.matmul(rd2_ps, ones_row, rd2_row, start=True, stop=True)
        RD2 = singles.tile([P, B], f32, tag="RD2")
        nc.scalar.copy(out=RD2, in_=rd2_ps)
        nc.vector.tensor_mul(ws3, ws3, bcast_inner(RD2, R))

        nc.sync.dma_start(
            out=out_ap.rearrange("b (p r) -> p b r", r=R),
            in_=ws.rearrange("p (b r) -> p b r", r=R),
        )


def _get_nc():
    if "nc" in _NC_CACHE:
        return _NC_CACHE["nc"]
    from concourse import bacc, mybir

    f32 = mybir.dt.float32
    nc = bacc.Bacc("TRN2", debug=False, num_devices=NCORES)
    ins = {
        "memory": nc.dram_tensor("memory", [B, N, M], f32, kind="ExternalInput").ap(),
        "k": nc.dram_tensor("k", [B, M], f32, kind="ExternalInput").ap(),
        "beta": nc.dram_tensor("beta", [B, 1], f32, kind="ExternalInput").ap(),
        "prev_w": nc.dram_tensor("prev_w", [B, N], f32, kind="ExternalInput").ap(),
        "g": nc.dram_tensor("g", [B, 1], f32, kind="ExternalInput").ap(),
        "s": nc.dram_tensor("s", [B, 3], f32, kind="ExternalInput").ap(),
        "gamma": nc.dram_tensor("gamma", [B, 1], f32, kind="ExternalInput").ap(),
    }
    out_ap = nc.dram_tensor("out", [B, N], f32, kind="ExternalOutput").ap()
    _build_body(nc, out_ap, ins)
    nc.finalize()
    _NC_CACHE["nc"] = nc
    return nc


def _shard_inputs(inputs):
    arrs = {
        name: np.ascontiguousarray(np.asarray(inputs[name], dtype=np.float32))
        for name in ("memory", "k", "beta", "prev_w", "g", "s", "gamma")
    }
    in_maps = []
    for c in range(NCORES):
        sl = slice(c * B, (c + 1) * B)
        in_maps.append({name: np.ascontiguousarray(a[sl]) for name, a in arrs.items()})
    return in_maps


def run(inputs, trace=False):
    from concourse.bass_utils import run_bass_kernel_spmd

    nc = _get_nc()
    in_maps = _shard_inputs(inputs)
    res = run_bass_kernel_spmd(
        nc, in_maps, core_ids=list(range(NCORES)), trace=trace,
        **({"trace_cores": [0]} if trace else {}),
    )
    out = np.concatenate([r["out"] for r in res.results], axis=0)
    return out, res


def kernel(**inputs):
    out, _ = run(inputs, trace=False)
    return out



# revision 5
# speedup vs baseline: 1.0188x; 1.0188x over previous
"""NTM addressing head (nn_HeadBase) Trainium2 Bass kernel.

Full-input contract: kernel(**inputs) takes the unsharded [256, ...] arrays,
shards batch-dim across 8 NeuronCores (pure data parallel), runs one SPMD Bass
program per core, and gathers the full [256, 4096] output.

Per-core layout (B=32 batches, N=4096, M=64):
  memory[b] is streamed as one [128, 2048] SBUF tile per batch with
  n = p*32 + r (partition p, free = (r, m)); 4 KB contiguous per partition.
  Phase A (per batch): prod = mem * k_bcast (DVE), dot = reduce over m
  (DVE), sq = mem^2 (ACT), ssq = reduce (DVE).
  Phase B (all batches fused as [128, 32b*32r] tiles): cosine sim, softmax
  (no max-subtract needed: beta*sim in (-1,1)), gated interpolation, 3-tap
  circular shift via shifted APs + partition-carry fixups, pow via exp/ln,
  final normalize.  Per-batch scalars are broadcast to [128, B] via K=1
  ones-matmuls on the TensorEngine.
"""

import numpy as np

B_FULL, N, M = 256, 4096, 64
NCORES = 8
B = B_FULL // NCORES   # 32 batches per core
P = 128                # SBUF partitions
R = N // P             # 32 rows per partition; n = p*R + r

_NC_CACHE = {}


def _build_body(nc, out_ap, ins):
    """Emit the kernel IR. ins: dict name->AP of DRAM inputs, out_ap: DRAM out."""
    from contextlib import ExitStack

    import concourse.bass as bass
    import concourse.tile as tile
    from concourse import mybir

    f32 = mybir.dt.float32
    Alu = mybir.AluOpType
    Act = mybir.ActivationFunctionType
    Ax = mybir.AxisListType
    AP = bass.AP

    bf16 = mybir.dt.bfloat16

    mem_ap = ins["memory"]   # [B, N, M]
    k_ap = ins["k"]          # [B, M]
    beta_ap = ins["beta"]    # [B, 1]
    pw_ap = ins["prev_w"]    # [B, N]
    g_ap = ins["g"]          # [B, 1]
    s_ap = ins["s"]          # [B, 3]
    gam_ap = ins["gamma"]    # [B, 1]

    def bcast_inner(ap2d, n):
        # [P, C] -> [P, C, n] with 0-stride inner dim
        return AP(ap2d.tensor, ap2d.offset, list(ap2d.ap) + [[0, n]])

    def bcast_mid(ap2d, n):
        # [P, C] -> [P, n, C] with 0-stride middle dim
        a = list(ap2d.ap)
        return AP(ap2d.tensor, ap2d.offset, [a[0], [0, n], a[1]])

    def row1(ap1d):
        # prepend a unit partition dim to a 1-d AP
        return AP(ap1d.tensor, ap1d.offset, [[0, 1]] + list(ap1d.ap))

    with tile.TileContext(nc) as tc, ExitStack() as ctx:
        singles = ctx.enter_context(tc.tile_pool(name="singles", bufs=1))
        mem_pool = ctx.enter_context(tc.tile_pool(name="mem", bufs=3))
        prod_pool = ctx.enter_context(tc.tile_pool(name="prod", bufs=2))
        big = ctx.enter_context(tc.tile_pool(name="big", bufs=1))
        ps = ctx.enter_context(tc.tile_pool(name="ps", bufs=2, space="PSUM"))
        ps_big = ctx.enter_context(tc.tile_pool(name="psbig", bufs=1, space="PSUM"))

        # ---- setup: constants, per-batch scalar rows on partition 0 ----
        ones_col = singles.tile([P, 1], f32, tag="ones_col")
        nc.vector.memset(ones_col, 1.0)
        ones_row = singles.tile([1, P], f32, tag="ones_row")
        nc.vector.memset(ones_row, 1.0)

        k_row = singles.tile([1, B * M], f32, tag="k_row")
        nc.sync.dma_start(out=k_row, in_=row1(k_ap.rearrange("b m -> (b m)")))
        b_row = singles.tile([1, B], f32, tag="b_row")
        nc.sync.dma_start(out=b_row, in_=row1(beta_ap.rearrange("b one -> (b one)")))
        g_row = singles.tile([1, B], f32, tag="g_row")
        nc.sync.dma_start(out=g_row, in_=row1(g_ap.rearrange("b one -> (b one)")))
        gm_row = singles.tile([1, B], f32, tag="gm_row")
        nc.sync.dma_start(out=gm_row, in_=row1(gam_ap.rearrange("b one -> (b one)")))
        s_row = singles.tile([1, 3 * B], f32, tag="s_row")
        nc.sync.dma_start(out=s_row, in_=row1(s_ap.rearrange("b i -> (b i)")))
        # s_i as [1, B] strided views (stride 3)
        s_perm = s_row.rearrange("p (b i) -> p i b", i=3)
        s_v = [s_perm[:, i, :] for i in range(3)]

        # k broadcast to all partitions: kb[p, b*M+m] = k[b, m].
        # Matmult can carry only ONE sync-wait; touch k_row on DVE first so
        # both matmul deps (ones_row memset + k data) ride the DVE semaphore.
        k_row2 = prod_pool.tile([1, B * M], f32, tag="pr")
        nc.vector.tensor_copy(k_row2, k_row)
        kb_psum = ps_big.tile([P, B * M], f32, tag="kb_psum")
        for j in range(0, B * M, 512):
            nc.tensor.matmul(
                kb_psum[:, j : j + 512], ones_row, k_row2[:, j : j + 512],
                start=True, stop=True,
            )
        kb = singles.tile([P, B * M], bf16, tag="kb")
        nc.scalar.copy(out=kb, in_=kb_psum)

        # knorm; bk = beta / knorm
        ksq_row = prod_pool.tile([1, B * M], f32, tag="pr")
        nc.vector.tensor_mul(ksq_row, k_row, k_row)
        ks_row = singles.tile([1, B], f32, tag="ks_row")
        nc.vector.tensor_reduce(
            out=ks_row, in_=ksq_row.rearrange("p (b m) -> p b m", m=M),
            axis=Ax.X, op=Alu.add,
        )
        kn_row = singles.tile([1, B], f32, tag="kn_row")
        nc.scalar.activation(out=kn_row, in_=ks_row, func=Act.Sqrt)
        rk_row = singles.tile([1, B], f32, tag="rk_row")
        nc.vector.reciprocal(out=rk_row, in_=kn_row)
        bk_row = singles.tile([1, B], f32, tag="bk_row")
        nc.vector.tensor_mul(bk_row, b_row, rk_row)

        # omg = 1 - g
        omg_row = singles.tile([1, B], f32, tag="omg_row")
        nc.vector.tensor_scalar(
            out=omg_row, in0=g_row, scalar1=-1.0, scalar2=1.0,
            op0=Alu.mult, op1=Alu.add,
        )

        # broadcast round 1: [bk, omg, s0, s1, s2, gamma] -> [P, 6*B]
        NSC = 6
        asm1 = singles.tile([1, NSC * B], f32, tag="asm1")
        for i, src in enumerate([bk_row, omg_row, s_v[0], s_v[1], s_v[2], gm_row]):
            nc.vector.tensor_copy(asm1[:, i * B : (i + 1) * B], src)
        bc1_ps = ps.tile([P, NSC * B], f32, tag="mm")
        nc.tensor.matmul(bc1_ps, ones_row, asm1, start=True, stop=True)
        BC1 = singles.tile([P, NSC * B], f32, tag="BC1")
        nc.scalar.copy(out=BC1, in_=bc1_ps)
        BK = BC1[:, 0 * B : 1 * B]
        OMG = BC1[:, 1 * B : 2 * B]
        S0 = BC1[:, 2 * B : 3 * B]
        S1 = BC1[:, 3 * B : 4 * B]
        S2 = BC1[:, 4 * B : 5 * B]
        GAM = BC1[:, 5 * B : 6 * B]

        # prev_w big tile [P, B*R] in one permuted-AP DMA (128B inner runs)
        pw = big.tile([P, B * R], f32, tag="pw")
        nc.sync.dma_start(
            out=pw.rearrange("p (b r) -> p b r", r=R),
            in_=pw_ap.rearrange("b (p r) -> p b r", r=R),
        )

        # ---- phase A: stream memory in CB-batch chunks ----
        # memory is cast fp32->bf16 during the DMA (SWDGE).  The k-multiply
        # runs as a bf16 tensor_tensor on DVE (2x mode) for most chunks, on
        # GpSimd for a few to balance load.  The m=64 reductions use pairwise
        # bf16 add-trees (2x) instead of tensor_reduce (stuck at 1x); the last
        # three levels are fp32 for accuracy.  Square runs in place on ACT.
        CB = 4  # batches per chunk
        dot = big.tile([P, B * R], f32, tag="dot")
        ssq = big.tile([P, B * R], f32, tag="ssq")
        tree = ctx.enter_context(tc.tile_pool(name="tree", bufs=3))

        def unit(ap3):
            return AP(ap3.tensor, ap3.offset, list(ap3.ap) + [[1, 1]])

        def mtree(src4, out3, tag):
            # src4 [P, CB, R, 64] bf16 -> out3 [P, CB, R] f32 (sum over m).
            # Pairwise halving in place (bf16 2x mode) down to 4 partials,
            # then an fp32 tail for accuracy.
            w = M // 2
            while w >= 4:
                nc.vector.tensor_add(
                    out=src4[:, :, :, 0:w], in0=src4[:, :, :, 0:w],
                    in1=src4[:, :, :, w : 2 * w],
                )
                w //= 2
            t = tree.tile([P, CB * R * 2], f32, tag=f"{tag}5")
            t4 = t.rearrange("p (b r h) -> p b r h", b=CB, h=2)
            nc.vector.tensor_add(
                out=t4, in0=src4[:, :, :, 0:2], in1=src4[:, :, :, 2:4]
            )
            nc.vector.tensor_add(
                out=unit(out3), in0=t4[:, :, :, 0:1], in1=t4[:, :, :, 1:2]
            )

        GPS_CHUNKS = (1, 4, 6)  # chunks whose multiply runs on GpSimd
        for c in range(B // CB):
            b0 = c * CB
            mt = mem_pool.tile([P, CB * R * M], bf16, tag="mt")
            nc.gpsimd.dma_start(
                out=mt.rearrange("p (b f) -> p b f", b=CB),
                in_=mem_ap[b0 : b0 + CB].rearrange(
                    "b (p r) m -> p b (r m)", p=P
                ),
            )
            mt4 = mt.rearrange("p (b r m) -> p b r m", b=CB, m=M)
            pr = prod_pool.tile([P, CB * R * M], bf16, tag="pr")
            pr4 = pr.rearrange("p (b r m) -> p b r m", b=CB, m=M)
            kbc = kb[:, b0 * M : (b0 + CB) * M]  # [P, CB*M]
            kb4 = AP(
                kbc.tensor, kbc.offset,
                [kbc.ap[0], [M, CB], [0, R], [1, M]],
            )
            if c in GPS_CHUNKS:
                nc.gpsimd.tensor_tensor(out=pr4, in0=mt4, in1=kb4, op=Alu.mult)
            else:
                nc.vector.tensor_mul(pr4, mt4, kb4)
            mtree(pr4, dot[:, b0 * R : (b0 + CB) * R].rearrange(
                "p (b r) -> p b r", b=CB), "d")
            # square mt in place (ACT); Tile orders it after the mult
            nc.scalar.square(out=mt, in_=mt)
            mtree(mt4, ssq[:, b0 * R : (b0 + CB) * R].rearrange(
                "p (b r) -> p b r", b=CB), "s")

        # ---- phase B ----
        def v3(t):
            return t.rearrange("p (b r) -> p b r", r=R)

        # rstd = 1/sqrt(ssq_avg)
        mn = big.tile([P, B * R], f32, tag="mn")
        nc.scalar.activation(out=mn, in_=ssq, func=Act.Sqrt)
        scr = prod_pool.tile([P, B * R], f32, tag="pr")
        nc.vector.reciprocal_approx_accurate(out=ssq, in_=mn, scratch=scr)

        # a = (8*beta/knorm) * dot_avg * rstd
        nc.vector.tensor_mul(dot, dot, ssq)
        nc.vector.tensor_mul(v3(dot), v3(dot), bcast_inner(BK, R))

        # e = exp(a)
        e = big.tile([P, B * R], f32, tag="e")
        nc.scalar.activation(out=e, in_=dot, func=Act.Exp)

        # denom per batch; gd = g/denom
        cs = singles.tile([P, B], f32, tag="cs")
        nc.vector.tensor_reduce(out=cs, in_=v3(e), axis=Ax.X, op=Alu.add)
        den_ps = ps.tile([1, B], f32, tag="mm")
        nc.tensor.matmul(den_ps, ones_col, cs, start=True, stop=True)
        rden_row = singles.tile([1, B], f32, tag="rden_row")
        nc.vector.reciprocal(out=rden_row, in_=den_ps)
        gd_row = singles.tile([1, B], f32, tag="gd_row")
        nc.vector.tensor_mul(gd_row, rden_row, g_row)
        gd_ps = ps.tile([P, B], f32, tag="mm")
        nc.tensor.matmul(gd_ps, ones_row, gd_row, start=True, stop=True)
        GD = singles.tile([P, B], f32, tag="GD")
        nc.scalar.copy(out=GD, in_=gd_ps)

        # wg = e*gd + pw*omg   (in place into e)
        nc.vector.tensor_mul(v3(e), v3(e), bcast_inner(GD, R))
        nc.vector.tensor_mul(v3(pw), v3(pw), bcast_inner(OMG, R))
        nc.vector.tensor_add(out=e, in0=e, in1=pw)

        # circular 3-tap shift: ws[n] = s1*wg[n] + s0*wg[n-1] + s2*wg[n+1]
        ws = big.tile([P, B * R], f32, tag="ws")
        ta = prod_pool.tile([P, B * R], f32, tag="pr")
        tb = prod_pool.tile([P, B * R], f32, tag="pr")
        wg3, ws3, ta3, tb3 = v3(e), v3(ws), v3(ta), v3(tb)
        nc.vector.tensor_mul(ws3, wg3, bcast_inner(S1, R))
        nc.vector.tensor_mul(ta3, wg3, bcast_inner(S0, R))
        nc.vector.tensor_mul(tb3, wg3, bcast_inner(S2, R))
        nc.vector.tensor_add(
            out=ws3[:, :, 1:R], in0=ws3[:, :, 1:R], in1=ta3[:, :, 0 : R - 1]
        )
        nc.vector.tensor_add(
            out=ws3[:, :, 0 : R - 1], in0=ws3[:, :, 0 : R - 1], in1=tb3[:, :, 1:R]
        )
        # partition carries: engines need 32-aligned start partitions, so the
        # +-1 partition rotation goes through small SBUF->SBUF DMAs.
        # tmp_dn[p] = ta[(p-1) mod P, :, R-1];  tmp_up[p] = tb[(p+1) mod P, :, 0]
        tmp_dn = singles.tile([P, B], f32, tag="tmp_dn")
        nc.sync.dma_start(out=tmp_dn[1:P, :], in_=ta3[0 : P - 1, :, R - 1 : R])
        nc.sync.dma_start(out=tmp_dn[0:1, :], in_=ta3[P - 1 : P, :, R - 1 : R])
        tmp_up = singles.tile([P, B], f32, tag="tmp_up")
        nc.sync.dma_start(out=tmp_up[0 : P - 1, :], in_=tb3[1:P, :, 0:1])
        nc.sync.dma_start(out=tmp_up[P - 1 : P, :], in_=tb3[0:1, :, 0:1])
        nc.vector.tensor_add(
            out=ws3[:, :, 0:1], in0=ws3[:, :, 0:1], in1=bcast_inner(tmp_dn, 1)
        )
        nc.vector.tensor_add(
            out=ws3[:, :, R - 1 : R], in0=ws3[:, :, R - 1 : R],
            in1=bcast_inner(tmp_up, 1),
        )

        # w_pow = ws ** gamma = exp(gamma * ln(ws))
        nc.scalar.activation(out=ws, in_=ws, func=Act.Ln)
        nc.vector.tensor_mul(ws3, ws3, bcast_inner(GAM, R))
        nc.scalar.activation(out=ws, in_=ws, func=Act.Exp)

        # normalize: out = w_pow / (sum + 1e-16)
        cs2 = singles.tile([P, B], f32, tag="cs2")
        nc.vector.tensor_reduce(out=cs2, in_=ws3, axis=Ax.X, op=Alu.add)
        d2_ps = ps.tile([1, B], f32, tag="mm")
        nc.tensor.matmul(d2_ps, ones_col, cs2, start=True, stop=True)
        d2_row = singles.tile([1, B], f32, tag="d2_row")
        nc.vector.tensor_scalar_add(out=d2_row, in0=d2_ps, scalar1=1e-16)
        rd2_row = singles.tile([1, B], f32, tag="rd2_row")
        nc.vector.reciprocal(out=rd2_row, in_=d2_row)
        rd2_ps = ps.tile([P, B], f32, tag="mm")
        nc.tensor.matmul(rd2_ps, ones_row, rd2_row, start=True, stop=True)
        RD2 = singles.tile([P, B], f32, tag="RD2")
        nc.scalar.copy(out=RD2, in_=rd2_ps)
        nc.vector.tensor_mul(ws3, ws3, bcast_inner(RD2, R))

        nc.sync.dma_start(
            out=out_ap.rearrange("b (p r) -> p b r", r=R),
            in_=ws.rearrange("p (b r) -> p b r", r=R),
        )


def _get_nc():
    if "nc" in _NC_CACHE:
        return _NC_CACHE["nc"]
    from concourse import bacc, mybir

    f32 = mybir.dt.float32
    nc = bacc.Bacc("TRN2", debug=False, num_devices=NCORES)
    ins = {
        "memory": nc.dram_tensor("memory", [B, N, M], f32, kind="ExternalInput").ap(),
        "k": nc.dram_tensor("k", [B, M], f32, kind="ExternalInput").ap(),
        "beta": nc.dram_tensor("beta", [B, 1], f32, kind="ExternalInput").ap(),
        "prev_w": nc.dram_tensor("prev_w", [B, N], f32, kind="ExternalInput").ap(),
        "g": nc.dram_tensor("g", [B, 1], f32, kind="ExternalInput").ap(),
        "s": nc.dram_tensor("s", [B, 3], f32, kind="ExternalInput").ap(),
        "gamma": nc.dram_tensor("gamma", [B, 1], f32, kind="ExternalInput").ap(),
    }
    out_ap = nc.dram_tensor("out", [B, N], f32, kind="ExternalOutput").ap()
    _build_body(nc, out_ap, ins)
    nc.finalize()
    _NC_CACHE["nc"] = nc
    return nc


def _shard_inputs(inputs):
    arrs = {
        name: np.ascontiguousarray(np.asarray(inputs[name], dtype=np.float32))
        for name in ("memory", "k", "beta", "prev_w", "g", "s", "gamma")
    }
    in_maps = []
    for c in range(NCORES):
        sl = slice(c * B, (c + 1) * B)
        in_maps.append({name: np.ascontiguousarray(a[sl]) for name, a in arrs.items()})
    return in_maps


def run(inputs, trace=False):
    from concourse.bass_utils import run_bass_kernel_spmd

    nc = _get_nc()
    in_maps = _shard_inputs(inputs)
    res = run_bass_kernel_spmd(
        nc, in_maps, core_ids=list(range(NCORES)), trace=trace,
        **({"trace_cores": [0]} if trace else {}),
    )
    out = np.concatenate([r["out"] for r in res.results], axis=0)
    return out, res


def kernel(**inputs):
    out, _ = run(inputs, trace=False)
    return out



# revision 13
# speedup vs baseline: 1.6277x; 1.5977x over previous
"""NTM addressing head (nn_HeadBase) Trainium2 Bass kernel.

Full-input contract: kernel(**inputs) takes the unsharded [256, ...] arrays,
shards batch-dim across 8 NeuronCores (pure data parallel), runs one SPMD Bass
program per core, and gathers the full [256, 4096] output.

Per-core layout (B=32 batches, N=4096, M=64):
  memory[b] is streamed as one [128, 2048] SBUF tile per batch with
  n = p*32 + r (partition p, free = (r, m)); 4 KB contiguous per partition.
  Phase A (per batch): prod = mem * k_bcast (DVE), dot = reduce over m
  (DVE), sq = mem^2 (ACT), ssq = reduce (DVE).
  Phase B (all batches fused as [128, 32b*32r] tiles): cosine sim, softmax
  (no max-subtract needed: beta*sim in (-1,1)), gated interpolation, 3-tap
  circular shift via shifted APs + partition-carry fixups, pow via exp/ln,
  final normalize.  Per-batch scalars are broadcast to [128, B] via K=1
  ones-matmuls on the TensorEngine.
"""

import numpy as np

B_FULL, N, M = 256, 4096, 64
NCORES = 8
B = B_FULL // NCORES   # 32 batches per core
P = 128                # SBUF partitions
R = N // P             # 32 rows per partition; n = p*R + r

_NC_CACHE = {}


def _build_body(nc, out_ap, ins):
    """Emit the kernel IR. ins: dict name->AP of DRAM inputs, out_ap: DRAM out."""
    from contextlib import ExitStack

    import concourse.bass as bass
    import concourse.tile as tile
    from concourse import mybir

    f32 = mybir.dt.float32
    Alu = mybir.AluOpType
    Act = mybir.ActivationFunctionType
    Ax = mybir.AxisListType
    AP = bass.AP

    bf16 = mybir.dt.bfloat16

    mem_ap = ins["memory"]   # [B, N, M]
    k_ap = ins["k"]          # [B, M]
    beta_ap = ins["beta"]    # [B, 1]
    pw_ap = ins["prev_w"]    # [B, N]
    g_ap = ins["g"]          # [B, 1]
    s_ap = ins["s"]          # [B, 3]
    gam_ap = ins["gamma"]    # [B, 1]

    def bcast_inner(ap2d, n):
        # [P, C] -> [P, C, n] with 0-stride inner dim
        return AP(ap2d.tensor, ap2d.offset, list(ap2d.ap) + [[0, n]])

    def bcast_mid(ap2d, n):
        # [P, C] -> [P, n, C] with 0-stride middle dim
        a = list(ap2d.ap)
        return AP(ap2d.tensor, ap2d.offset, [a[0], [0, n], a[1]])

    def row1(ap1d):
        # prepend a unit partition dim to a 1-d AP
        return AP(ap1d.tensor, ap1d.offset, [[0, 1]] + list(ap1d.ap))

    with tile.TileContext(nc) as tc, ExitStack() as ctx:
        singles = ctx.enter_context(tc.tile_pool(name="singles", bufs=1))
        mem_pool = ctx.enter_context(tc.tile_pool(name="mem", bufs=5))
        prod_pool = ctx.enter_context(tc.tile_pool(name="prod", bufs=3))
        big = ctx.enter_context(tc.tile_pool(name="big", bufs=1))
        ps = ctx.enter_context(tc.tile_pool(name="ps", bufs=2, space="PSUM"))
        ps_big = ctx.enter_context(tc.tile_pool(name="psbig", bufs=1, space="PSUM"))

        # ---- setup: constants, per-batch scalar rows on partition 0 ----
        ones_col = singles.tile([P, 1], f32, tag="ones_col")
        nc.vector.memset(ones_col, 1.0)
        ones_row = singles.tile([1, P], f32, tag="ones_row")
        nc.vector.memset(ones_row, 1.0)

        k_row = singles.tile([1, B * M], f32, tag="k_row")
        nc.sync.dma_start(out=k_row, in_=row1(k_ap.rearrange("b m -> (b m)")))
        b_row = singles.tile([1, B], f32, tag="b_row")
        nc.sync.dma_start(out=b_row, in_=row1(beta_ap.rearrange("b one -> (b one)")))
        g_row = singles.tile([1, B], f32, tag="g_row")
        nc.sync.dma_start(out=g_row, in_=row1(g_ap.rearrange("b one -> (b one)")))
        gm_row = singles.tile([1, B], f32, tag="gm_row")
        nc.sync.dma_start(out=gm_row, in_=row1(gam_ap.rearrange("b one -> (b one)")))
        s_row = singles.tile([1, 3 * B], f32, tag="s_row")
        nc.sync.dma_start(out=s_row, in_=row1(s_ap.rearrange("b i -> (b i)")))
        # s_i as [1, B] strided views (stride 3)
        s_perm = s_row.rearrange("p (b i) -> p i b", i=3)
        s_v = [s_perm[:, i, :] for i in range(3)]

        # k broadcast to all partitions: kb[p, b*M+m] = k[b, m].
        # Matmult can carry only ONE sync-wait; touch k_row on DVE first so
        # both matmul deps (ones_row memset + k data) ride the DVE semaphore.
        k_row2 = prod_pool.tile([1, B * M], f32, tag="pr")
        nc.vector.tensor_copy(k_row2, k_row)
        kb_psum = ps_big.tile([P, B * M], f32, tag="kb_psum")
        for j in range(0, B * M, 512):
            nc.tensor.matmul(
                kb_psum[:, j : j + 512], ones_row, k_row2[:, j : j + 512],
                start=True, stop=True,
            )
        kb = singles.tile([P, B * M], bf16, tag="kb")
        nc.scalar.copy(out=kb, in_=kb_psum)

        # knorm; bk = beta / knorm
        ksq_row = prod_pool.tile([1, B * M], f32, tag="pr")
        nc.vector.tensor_mul(ksq_row, k_row, k_row)
        ks_row = singles.tile([1, B], f32, tag="ks_row")
        nc.vector.tensor_reduce(
            out=ks_row, in_=ksq_row.rearrange("p (b m) -> p b m", m=M),
            axis=Ax.X, op=Alu.add,
        )
        kn_row = singles.tile([1, B], f32, tag="kn_row")
        nc.scalar.activation(out=kn_row, in_=ks_row, func=Act.Sqrt)
        rk_row = singles.tile([1, B], f32, tag="rk_row")
        nc.vector.reciprocal(out=rk_row, in_=kn_row)
        bk_row = singles.tile([1, B], f32, tag="bk_row")
        nc.vector.tensor_mul(bk_row, b_row, rk_row)

        # omg = 1 - g
        omg_row = singles.tile([1, B], f32, tag="omg_row")
        nc.vector.tensor_scalar(
            out=omg_row, in0=g_row, scalar1=-1.0, scalar2=1.0,
            op0=Alu.mult, op1=Alu.add,
        )

        # broadcast round 1: [bk, omg, s0, s1, s2, gamma] -> [P, 6*B]
        NSC = 6
        asm1 = singles.tile([1, NSC * B], f32, tag="asm1")
        for i, src in enumerate([bk_row, omg_row, s_v[0], s_v[1], s_v[2], gm_row]):
            nc.vector.tensor_copy(asm1[:, i * B : (i + 1) * B], src)
        bc1_ps = ps.tile([P, NSC * B], f32, tag="mm")
        nc.tensor.matmul(bc1_ps, ones_row, asm1, start=True, stop=True)
        BC1 = singles.tile([P, NSC * B], f32, tag="BC1")
        nc.scalar.copy(out=BC1, in_=bc1_ps)
        BK = BC1[:, 0 * B : 1 * B]
        OMG = BC1[:, 1 * B : 2 * B]
        S0 = BC1[:, 2 * B : 3 * B]
        S1 = BC1[:, 3 * B : 4 * B]
        S2 = BC1[:, 4 * B : 5 * B]
        GAM = BC1[:, 5 * B : 6 * B]

        # prev_w big tile [P, B*R] in one permuted-AP DMA (128B inner runs)
        pw = big.tile([P, B * R], f32, tag="pw")
        nc.sync.dma_start(
            out=pw.rearrange("p (b r) -> p b r", r=R),
            in_=pw_ap.rearrange("b (p r) -> p b r", r=R),
        )

        # ---- phase A: stream memory in CB-batch chunks ----
        # memory is cast fp32->bf16 during the DMA (SWDGE).  The k-multiply
        # runs as a bf16 tensor_tensor on DVE (2x mode) for most chunks, on
        # GpSimd for a few to balance load.  The m=64 reductions use pairwise
        # bf16 add-trees (2x) instead of tensor_reduce (stuck at 1x); the last
        # three levels are fp32 for accuracy.  Square runs in place on ACT.
        CB = 4  # batches per chunk
        dot = big.tile([P, B * R], f32, tag="dot")
        ssq = big.tile([P, B * R], f32, tag="ssq")
        tree = ctx.enter_context(tc.tile_pool(name="tree", bufs=3))

        def unit(ap3):
            return AP(ap3.tensor, ap3.offset, list(ap3.ap) + [[1, 1]])

        def mtree(src4, out3, tag):
            # src4 [P, CB, R, 64] bf16 -> out3 [P, CB, R] f32 (sum over m).
            # Pairwise halving in place (bf16 2x mode) down to 4 partials,
            # then an fp32 tail for accuracy.
            w = M // 2
            while w >= 4:
                nc.vector.tensor_add(
                    out=src4[:, :, :, 0:w], in0=src4[:, :, :, 0:w],
                    in1=src4[:, :, :, w : 2 * w],
                )
                w //= 2
            t = tree.tile([P, CB * R * 2], f32, tag=f"{tag}5")
            t4 = t.rearrange("p (b r h) -> p b r h", b=CB, h=2)
            nc.vector.tensor_add(
                out=t4, in0=src4[:, :, :, 0:2], in1=src4[:, :, :, 2:4]
            )
            nc.vector.tensor_add(
                out=unit(out3), in0=t4[:, :, :, 0:1], in1=t4[:, :, :, 1:2]
            )

        # circular partition-shift matrices for the 3-tap carry (built once):
        # Sdn: out[i,:] = in[(i-1) mod P, :];  Sup: out[i,:] = in[(i+1) mod P, :]
        i32 = mybir.dt.int32
        col_idx = singles.tile([P, P], i32, tag="col_idx")
        nc.gpsimd.iota(col_idx, pattern=[[1, P]], base=0, channel_multiplier=0)
        def bc_row(ap2d, n):
            # [P, 1] -> [P, n] with 0-stride free dim
            return AP(ap2d.tensor, ap2d.offset, [list(ap2d.ap)[0], [0, n]])

        def shift_mat(tag, base):
            # S[p, i] = 1 if i == (p + base) mod P; wrap via second compare
            # (engines cannot memset a single unaligned partition).
            ra = singles.tile([P, 1], i32, tag=f"{tag}_ra")
            nc.gpsimd.iota(ra, pattern=[[0, 1]], base=base, channel_multiplier=1)
            rb = singles.tile([P, 1], i32, tag=f"{tag}_rb")
            nc.gpsimd.iota(rb, pattern=[[0, 1]],
                           base=base - P if base > 0 else base + P,
                           channel_multiplier=1)
            sa = singles.tile([P, P], bf16, tag=f"{tag}_a")
            nc.vector.tensor_tensor(
                out=sa, in0=col_idx, in1=bc_row(ra, P), op=Alu.is_equal)
            sb = singles.tile([P, P], bf16, tag=f"{tag}_b")
            nc.vector.tensor_tensor(
                out=sb, in0=col_idx, in1=bc_row(rb, P), op=Alu.is_equal)
            nc.vector.tensor_add(out=sa, in0=sa, in1=sb)
            return sa

        sdn = shift_mat("sdn", 1)
        sup = shift_mat("sup", -1)

        for c in range(B // CB):
            b0 = c * CB
            mt = mem_pool.tile([P, CB * R * M], bf16, tag="mt")
            nc.gpsimd.dma_start(
                out=mt.rearrange("p (b f) -> p b f", b=CB),
                in_=mem_ap[b0 : b0 + CB].rearrange(
                    "b (p r) m -> p b (r m)", p=P
                ),
            )
            mt4 = mt.rearrange("p (b r m) -> p b r m", b=CB, m=M)
            pr = prod_pool.tile([P, CB * R * M], bf16, tag="pr")
            pr4 = pr.rearrange("p (b r m) -> p b r m", b=CB, m=M)
            kbc = kb[:, b0 * M : (b0 + CB) * M]  # [P, CB*M]
            kb4 = AP(
                kbc.tensor, kbc.offset,
                [kbc.ap[0], [M, CB], [0, R], [1, M]],
            )
            nc.vector.tensor_mul(pr4, mt4, kb4)
            mtree(pr4, dot[:, b0 * R : (b0 + CB) * R].rearrange(
                "p (b r) -> p b r", b=CB), "d")
            # square mt in place (ACT); Tile orders it after the mult
            nc.scalar.square(out=mt, in_=mt)
            mtree(mt4, ssq[:, b0 * R : (b0 + CB) * R].rearrange(
                "p (b r) -> p b r", b=CB), "s")

        # ---- phase B ----
        def v3(t):
            return t.rearrange("p (b r) -> p b r", r=R)

        # rstd = 1/sqrt(ssq_avg)
        mn = big.tile([P, B * R], f32, tag="mn")
        nc.scalar.activation(out=mn, in_=ssq, func=Act.Sqrt)
        scr = prod_pool.tile([P, B * R], f32, tag="pr")
        nc.vector.reciprocal_approx_accurate(out=ssq, in_=mn, scratch=scr)

        # a = (8*beta/knorm) * dot_avg * rstd
        nc.vector.tensor_mul(dot, dot, ssq)
        nc.vector.tensor_mul(v3(dot), v3(dot), bcast_inner(BK, R))

        # e = exp(a)
        e = big.tile([P, B * R], f32, tag="e")
        nc.scalar.activation(out=e, in_=dot, func=Act.Exp)

        # denom per batch; gd = g/denom
        cs = singles.tile([P, B], f32, tag="cs")
        nc.vector.tensor_reduce(out=cs, in_=v3(e), axis=Ax.X, op=Alu.add)
        den_ps = ps.tile([1, B], f32, tag="mm")
        nc.tensor.matmul(den_ps, ones_col, cs, start=True, stop=True)
        rden_row = singles.tile([1, B], f32, tag="rden_row")
        nc.vector.reciprocal(out=rden_row, in_=den_ps)
        gd_row = singles.tile([1, B], f32, tag="gd_row")
        nc.vector.tensor_mul(gd_row, rden_row, g_row)
        gd_ps = ps.tile([P, B], f32, tag="mm")
        nc.tensor.matmul(gd_ps, ones_row, gd_row, start=True, stop=True)
        GD = singles.tile([P, B], f32, tag="GD")
        nc.scalar.copy(out=GD, in_=gd_ps)

        # wg = e*gd + pw*omg   (in place into e)
        nc.vector.tensor_mul(v3(e), v3(e), bcast_inner(GD, R))
        nc.vector.tensor_mul(v3(pw), v3(pw), bcast_inner(OMG, R))
        nc.vector.tensor_add(out=e, in0=e, in1=pw)

        # circular 3-tap shift: ws[n] = s1*wg[n] + s0*wg[n-1] + s2*wg[n+1]
        ws = big.tile([P, B * R], f32, tag="ws")
        ta = prod_pool.tile([P, B * R], bf16, tag="pr")
        tb = prod_pool.tile([P, B * R], bf16, tag="pr")
        wg3, ws3, ta3, tb3 = v3(e), v3(ws), v3(ta), v3(tb)
        nc.vector.tensor_mul(ws3, wg3, bcast_inner(S1, R))
        nc.vector.tensor_mul(ta3, wg3, bcast_inner(S0, R))
        nc.vector.tensor_mul(tb3, wg3, bcast_inner(S2, R))
        # partition carries via circular-shift matmuls on the (idle) PE:
        # tmp_dn[p] = ta[(p-1) mod P, :, R-1];  tmp_up[p] = tb[(p+1) mod P, :, 0]
        dn_ps = ps.tile([P, B], f32, tag="mm")
        nc.tensor.matmul(
            dn_ps, sdn,
            ta3[:, :, R - 1 : R].rearrange("p b one -> p (b one)"),
            start=True, stop=True,
        )
        up_ps = ps.tile([P, B], f32, tag="mm")
        nc.tensor.matmul(
            up_ps, sup,
            tb3[:, :, 0:1].rearrange("p b one -> p (b one)"),
            start=True, stop=True,
        )
        nc.vector.tensor_add(
            out=ws3[:, :, 1:R], in0=ws3[:, :, 1:R], in1=ta3[:, :, 0 : R - 1]
        )
        nc.vector.tensor_add(
            out=ws3[:, :, 0 : R - 1], in0=ws3[:, :, 0 : R - 1], in1=tb3[:, :, 1:R]
        )
        nc.vector.tensor_add(
            out=ws3[:, :, 0:1], in0=ws3[:, :, 0:1], in1=bcast_inner(dn_ps, 1)
        )
        nc.vector.tensor_add(
            out=ws3[:, :, R - 1 : R], in0=ws3[:, :, R - 1 : R],
            in1=bcast_inner(up_ps, 1),
        )

        # w_pow = ws ** gamma = exp(gamma * ln(ws))
        nc.scalar.activation(out=ws, in_=ws, func=Act.Ln)
        nc.vector.tensor_mul(ws3, ws3, bcast_inner(GAM, R))
        nc.scalar.activation(out=ws, in_=ws, func=Act.Exp)

        # normalize: out = w_pow / (sum + 1e-16)
        cs2 = singles.tile([P, B], f32, tag="cs2")
        nc.vector.tensor_reduce(out=cs2, in_=ws3, axis=Ax.X, op=Alu.add)
        d2_ps = ps.tile([1, B], f32, tag="mm")
        nc.tensor.matmul(d2_ps, ones_col, cs2, start=True, stop=True)
        d2_row = singles.tile([1, B], f32, tag="d2_row")
        nc.vector.tensor_scalar_add(out=d2_row, in0=d2_ps, scalar1=1e-16)
        rd2_row = singles.tile([1, B], f32, tag="rd2_row")
        nc.vector.reciprocal(out=rd2_row, in_=d2_row)
        rd2_ps = ps.tile([P, B], f32, tag="mm")
        nc.tensor.matmul(rd2_ps, ones_row, rd2_row, start=True, stop=True)
        RD2 = singles.tile([P, B], f32, tag="RD2")
        nc.scalar.copy(out=RD2, in_=rd2_ps)
        nc.vector.tensor_mul(ws3, ws3, bcast_inner(RD2, R))

        nc.sync.dma_start(
            out=out_ap.rearrange("b (p r) -> p b r", r=R),
            in_=ws.rearrange("p (b r) -> p b r", r=R),
        )


def _get_nc():
    if "nc" in _NC_CACHE:
        return _NC_CACHE["nc"]
    from concourse import bacc, mybir

    f32 = mybir.dt.float32
    nc = bacc.Bacc("TRN2", debug=False, num_devices=NCORES)
    ins = {
        "memory": nc.dram_tensor("memory", [B, N, M], f32, kind="ExternalInput").ap(),
        "k": nc.dram_tensor("k", [B, M], f32, kind="ExternalInput").ap(),
        "beta": nc.dram_tensor("beta", [B, 1], f32, kind="ExternalInput").ap(),
        "prev_w": nc.dram_tensor("prev_w", [B, N], f32, kind="ExternalInput").ap(),
        "g": nc.dram_tensor("g", [B, 1], f32, kind="ExternalInput").ap(),
        "s": nc.dram_tensor("s", [B, 3], f32, kind="ExternalInput").ap(),
        "gamma": nc.dram_tensor("gamma", [B, 1], f32, kind="ExternalInput").ap(),
    }
    out_ap = nc.dram_tensor("out", [B, N], f32, kind="ExternalOutput").ap()
    _build_body(nc, out_ap, ins)
    nc.finalize()
    _NC_CACHE["nc"] = nc
    return nc


def _shard_inputs(inputs):
    arrs = {
        name: np.ascontiguousarray(np.asarray(inputs[name], dtype=np.float32))
        for name in ("memory", "k", "beta", "prev_w", "g", "s", "gamma")
    }
    in_maps = []
    for c in range(NCORES):
        sl = slice(c * B, (c + 1) * B)
        in_maps.append({name: np.ascontiguousarray(a[sl]) for name, a in arrs.items()})
    return in_maps


def run(inputs, trace=False):
    from concourse.bass_utils import run_bass_kernel_spmd

    nc = _get_nc()
    in_maps = _shard_inputs(inputs)
    res = run_bass_kernel_spmd(
        nc, in_maps, core_ids=list(range(NCORES)), trace=trace,
        **({"trace_cores": [0]} if trace else {}),
    )
    out = np.concatenate([r["out"] for r in res.results], axis=0)
    return out, res


def kernel(**inputs):
    out, _ = run(inputs, trace=False)
    return out

